# revision 17
# baseline (speedup 1.0000x reference)
"""Trainium2 Bass kernel for nn_CategoricalGumbelSoftmax.

Math (per reference):
  hs = gene_emb @ W1s^T ; ht = gene_emb @ W1t^T (+b1 folded into ht)
  h[b,s,t,:] = relu(hs[b,s,:] + ht[b,t,:])
  z = h @ W2^T + b2 + (-log(-log u));  y = one_hot(argmax z);  M = y2 - y0

Sharding: src-gene axis split across 8 cores (64 src x 2 batches = 128
(b,s) instances per core). Weights replicated.

Device pipeline (per core):
  A: PE projects htT (d-part, t-free; b1 folded) and hsT; htT also goes
     to DRAM for replication; hs (negated + plain) to DRAM for gathers.
  R: htT replicated x8 along partitions -> htRep tiles (128 = 16d x 8
     instances, 512) via DMA.
  B: relu for 8-instance groups, partition = (i inst, l d-of-16).
     ACT groups: h = Relu(htRep + hsCol)  (exact relu w/ bias)
     DVE/GPS groups: h' = max(htRep, -hsCol) = h - hs (fast TT with a
     free-broadcast column; fp32r write). The missing +hs term is
     rank-1 in d and is added in stage E via hsw = W2 @ hs (exact).
  C: z' = blockdiag(W2)^T @ h on PE (fp32r, K = 8 inst x 16 d, M = 24),
     16 chunk-matmuls accumulate per group -> PSUM (24, 512).
  D: PSUM -> SBUF copy -> DMA to DRAM z planes [c, (b,s), t].
  E: gumbel (2x Ln on ACT), z~ = Z + mask*hsw + b2 - log(-log u),
     argmax via max + is_equal, M = y2 - y0.
"""
import numpy as np

import concourse.bacc as bacc
import concourse.mybir as mybir
from concourse.tile import TileContext
from concourse.bass_utils import run_bass_kernel_spmd

dt = mybir.dt
F32 = dt.float32
F32R = dt.float32r
AF = mybir.ActivationFunctionType
OP = mybir.AluOpType

B, G, D, C = 2, 512, 256, 3
N_CORES = 8
S = G // N_CORES          # 64 src genes per core per batch
P = B * S                 # 128 (b,s) instances per core
T = G                     # 512 targets
KC = D // 128             # 2 x 128 contraction chunks (stage A)
R = 8                     # instances packed per matmul column (stage C)
L = 128 // R              # 16 d-values per instance per column
NCH = D // L              # 16 K-chunks per group
NG = P // R               # 16 groups
MW = C * R                # 24 used output rows per group

# Per-group engine for the relu stage: 'a' = ACT (exact relu),
# 'v' = DVE / 'g' = GPSIMD (max-form, needs hsw correction in stage E).
GROUP_ENG = list("avavavavavavavav")


def build_program(b2_vals, sim_safe=False):
    nc = bacc.Bacc(None, target_bir_lowering=False)

    geT = nc.dram_tensor("geT", [B, D, T], F32, kind="ExternalInput")
    geS = nc.dram_tensor("geS", [B, D, S], F32, kind="ExternalInput")
    w1sT = nc.dram_tensor("w1sT", [D, D], F32, kind="ExternalInput")
    w1tT = nc.dram_tensor("w1tT", [D, D], F32, kind="ExternalInput")
    w2tp = nc.dram_tensor("w2tp", [D, 32], F32, kind="ExternalInput")
    w2blk = nc.dram_tensor("w2blk", [NCH, 128, 32], F32, kind="ExternalInput")
    b1c = nc.dram_tensor("b1c", [D, 1], F32, kind="ExternalInput")
    hswmask = nc.dram_tensor("hswmask", [P, 1], F32, kind="ExternalInput")
    u_pl = nc.dram_tensor("u_pl", [C, P, T], F32, kind="ExternalInput")
    y_pl = nc.dram_tensor("y_pl", [C, P, T], F32, kind="ExternalOutput")
    m_pl = nc.dram_tensor("m_pl", [P, T], F32, kind="ExternalOutput")

    with TileContext(nc) as tc:
        with (
            tc.tile_pool(name="wsb", bufs=1) as wsb,
            tc.tile_pool(name="hts", bufs=1) as hts,
            tc.tile_pool(name="hrep", bufs=4) as hrep,
            tc.tile_pool(name="hpool", bufs=8) as hpool,
            tc.tile_pool(name="hcolp", bufs=9) as hcolp,
            tc.tile_pool(name="zdrain", bufs=3) as zdrain,
            tc.tile_pool(name="epool", bufs=1) as epool,
            tc.tile_pool(name="ps", bufs=1, space="PSUM") as psp,
            tc.tile_pool(name="dscratch", bufs=1, space="DRAM") as dscratch,
        ):
            # ---- input loads ----
            ge_sb = [[wsb.tile([128, T], F32, name=f"ge{b}{k}") for k in range(KC)]
                     for b in range(B)]
            geS_sb = [[wsb.tile([128, S], F32, name=f"gs{b}{k}") for k in range(KC)]
                      for b in range(B)]
            w1s_sb = [[wsb.tile([128, 128], F32, name=f"w1s{k}{m}") for m in range(KC)]
                      for k in range(KC)]
            w1t_sb = [[wsb.tile([128, 128], F32, name=f"w1t{k}{m}") for m in range(KC)]
                      for k in range(KC)]
            w2_sb = [wsb.tile([128, 32], F32, name=f"w2f{k}") for k in range(KC)]
            w2b_sb = [wsb.tile([128, 32], F32, name=f"w2b{k}") for k in range(NCH)]
            b1_sb = [wsb.tile([128, 1], F32, name=f"b1{k}") for k in range(KC)]
            msk_sb = wsb.tile([P, 1], F32, name="msk")
            nc.sync.dma_start(out=msk_sb[:], in_=hswmask[:])
            for b in range(B):
                for k in range(KC):
                    nc.sync.dma_start(out=ge_sb[b][k][:].bitcast(F32R),
                                      in_=geT[b, 128 * k:128 * (k + 1), :].bitcast(F32R))
                    nc.sync.dma_start(out=geS_sb[b][k][:].bitcast(F32R),
                                      in_=geS[b, 128 * k:128 * (k + 1), :].bitcast(F32R))
            for k in range(KC):
                for m in range(KC):
                    nc.sync.dma_start(
                        out=w1s_sb[k][m][:].bitcast(F32R),
                        in_=w1sT[128 * k:128 * (k + 1), 128 * m:128 * (m + 1)].bitcast(F32R))
                    nc.sync.dma_start(
                        out=w1t_sb[k][m][:].bitcast(F32R),
                        in_=w1tT[128 * k:128 * (k + 1), 128 * m:128 * (m + 1)].bitcast(F32R))
                # fp32 copy of W2 for the exact hsw matmul
                nc.sync.dma_start(out=w2_sb[k][:],
                                  in_=w2tp[128 * k:128 * (k + 1), :])
                nc.sync.dma_start(out=b1_sb[k][:],
                                  in_=b1c[128 * k:128 * (k + 1), :])
            for k in range(NCH):
                nc.sync.dma_start(out=w2b_sb[k][:].bitcast(F32R),
                                  in_=w2blk[k].bitcast(F32R))

            u_sb = [epool.tile([P, T], F32, name=f"u{c}") for c in range(C)]
            for c in range(C):
                nc.sync.dma_start(out=u_sb[c][:], in_=u_pl[c])

            # ---- stage A: projections ----
            htT_sb = [[hts.tile([128, T], F32, name=f"ht{b}{m}") for m in range(KC)]
                      for b in range(B)]
            hsT_sb = [[hts.tile([128, S], F32, name=f"hs{b}{m}") for m in range(KC)]
                      for b in range(B)]
            htD = dscratch.tile([B, D, T], F32, name="htD")
            hsD2 = dscratch.tile([B, 2, S, D], F32, name="hsD2")  # [neg, pos]
            hsP_sb = [hts.tile([S, D], F32, name=f"hsP{b}") for b in range(B)]
            nhsP_sb = [hts.tile([S, D], F32, name=f"nhsP{b}") for b in range(B)]
            for b in range(B):
                for m in range(KC):
                    pht = psp.tile([128, T], F32, name="pht", tag="pz0")
                    for k in range(KC):
                        nc.tensor.matmul(pht[:], w1t_sb[k][m][:].bitcast(F32R),
                                         ge_sb[b][k][:].bitcast(F32R),
                                         start=(k == 0), stop=(k == KC - 1))
                    nc.scalar.activation(htT_sb[b][m][:], pht[:], AF.Identity,
                                         bias=b1_sb[m][:], scale=1.0)
                    nc.sync.dma_start(out=htD[b, 128 * m:128 * (m + 1), :],
                                      in_=htT_sb[b][m][:])
                    phs = psp.tile([128, S], F32, name="phs", tag="pz1")
                    for k in range(KC):
                        nc.tensor.matmul(phs[:], w1s_sb[k][m][:].bitcast(F32R),
                                         geS_sb[b][k][:].bitcast(F32R),
                                         start=(k == 0), stop=(k == KC - 1))
                    nc.vector.tensor_copy(hsT_sb[b][m][:], phs[:])
                    # hsP (s-part, d): swapped-operand projection
                    php = psp.tile([S, 128], F32, name="php", tag="pz2")
                    for k in range(KC):
                        nc.tensor.matmul(php[:], geS_sb[b][k][:].bitcast(F32R),
                                         w1s_sb[k][m][:].bitcast(F32R),
                                         start=(k == 0), stop=(k == KC - 1))
                    nc.vector.tensor_copy(hsP_sb[b][:, 128 * m:128 * (m + 1)],
                                          php[:])
                nc.vector.tensor_scalar(nhsP_sb[b][:], hsP_sb[b][:], -1.0,
                                        None, OP.mult)
                nc.sync.dma_start(out=hsD2[b, 0], in_=nhsP_sb[b][:])
                nc.sync.dma_start(out=hsD2[b, 1], in_=hsP_sb[b][:])

            # hsw[c, s] = sum_d W2[c,d] hs[s,d], exact fp32 (tiny matmul)
            hswD = dscratch.tile([C, P], F32, name="hswD")
            for b in range(B):
                phw = psp.tile([32, S], F32, name="phw", tag="pz3")
                for k in range(KC):
                    nc.tensor.matmul(phw[:], w2_sb[k][:], hsT_sb[b][k][:],
                                     start=(k == 0), stop=(k == KC - 1))
                hww = hts.tile([C, S], F32, name=f"hww{b}")
                nc.vector.tensor_copy(hww[:], phw[0:C, :])
                nc.sync.dma_start(out=hswD[:, b * S:(b + 1) * S], in_=hww[:])

            # ---- stages R+B+C+D ----
            z_t = dscratch.tile([C, P, T], F32, name="zt")
            for b in range(B):
                pz_tiles = [psp.tile([32, T], F32, name=f"pz{b}{sg}", tag=f"pz{sg}")
                            for sg in range(NG // B)]
                def load_col(sg, c16):
                    grp_ = b * (NG // B) + sg
                    sgn = 0 if GROUP_ENG[grp_] != "a" else 1
                    hcol = hcolp.tile([128, 1], F32,
                                      name=f"hc{b}{sg}{c16}", tag="hcol")
                    srcb = hsD2[b, sgn, sg * R:(sg + 1) * R,
                                c16 * L:(c16 + 1) * L]
                    if sim_safe:
                        for i in range(R):
                            nc.sync.dma_start(
                                out=hcol[i * L:(i + 1) * L, :],
                                in_=srcb[i].rearrange("(l one) -> l one",
                                                      one=1))
                    else:
                        nc.sync.dma_start(out=hcol[:], in_=srcb)
                    return hcol
                for c16 in range(NCH):
                    hr = hrep.tile([128, T], F32, name="hr", tag="hr")
                    src = htD[b, c16 * L:(c16 + 1) * L, :]
                    for rep in range(R):
                        nc.sync.dma_start(out=hr[rep * L:(rep + 1) * L, :],
                                          in_=src)
                    for sg in range(NG // B):
                        grp = b * (NG // B) + sg
                        eng = GROUP_ENG[grp]
                        h = hpool.tile([128, T], F32, name="h", tag="h")
                        hcol = load_col(sg, c16)
                        colbc = hcol[:].to_broadcast((128, T))
                        if eng == "a":
                            nc.scalar.activation(h[:].bitcast(F32R), hr[:],
                                                 AF.Relu,
                                                 bias=hcol[:],
                                                 scale=1.0)
                        elif eng == "v":
                            nc.vector.tensor_tensor(h[:].bitcast(F32R), hr[:],
                                                    colbc, OP.max)
                        else:
                            nc.gpsimd.tensor_tensor(h[:].bitcast(F32R), hr[:],
                                                    colbc, OP.max)
                        nc.tensor.matmul(pz_tiles[sg][0:MW, :],
                                         w2b_sb[c16][:, 0:MW].bitcast(F32R),
                                         h[:].bitcast(F32R),
                                         start=(c16 == 0), stop=(c16 == NCH - 1))
                for sg in range(NG // B):
                    grp = b * (NG // B) + sg
                    zsb = zdrain.tile([MW, T], F32, name="zsb", tag="zsb")
                    if sg % 2 == 0:
                        nc.vector.tensor_copy(zsb[:], pz_tiles[sg][0:MW, :])
                    else:
                        nc.scalar.copy(zsb[:], pz_tiles[sg][0:MW, :])
                    if sim_safe:
                        for i in range(R):
                            for c in range(C):
                                bs_i = grp * R + i
                                nc.sync.dma_start(
                                    out=z_t[c, bs_i:bs_i + 1, :],
                                    in_=zsb[C * i + c:C * i + c + 1, :])
                    else:
                        for c in range(C):
                            nc.sync.dma_start(
                                out=z_t[c, grp * R:(grp + 1) * R, :],
                                in_=zsb[c:c + C * (R - 1) + 1:C, :])

            # ---- stage E ----
            hswc = []
            for c in range(C):
                raw = epool.tile([P, 1], F32, name=f"hswraw{c}")
                nc.sync.dma_start(out=raw[:],
                                  in_=hswD[c].rearrange("(p one) -> p one", one=1))
                msked = epool.tile([P, 1], F32, name=f"hswm{c}")
                nc.vector.tensor_tensor(msked[:], raw[:], msk_sb[:], OP.mult)
                hswc.append(msked)
            q_sb = []
            for c in range(C):
                tln = epool.tile([P, T], F32, name=f"tln{c}")
                nc.scalar.activation(tln[:], u_sb[c][:], AF.Ln)
                q = epool.tile([P, T], F32, name=f"q{c}")
                nc.scalar.activation(q[:], tln[:], AF.Ln, scale=-1.0)
                q_sb.append(q)
            zt_sb = []
            for c in range(C):
                zc = epool.tile([P, T], F32, name=f"z{c}")
                nc.sync.dma_start(out=zc[:], in_=z_t[c])
                zhw = epool.tile([P, T], F32, name=f"zh{c}")
                nc.vector.tensor_tensor(zhw[:], zc[:],
                                        hswc[c][:].to_broadcast((P, T)), OP.add)
                ztc = epool.tile([P, T], F32, name=f"zt{c}")
                nc.vector.scalar_tensor_tensor(ztc[:], zhw[:], float(b2_vals[c]),
                                               q_sb[c][:], OP.add, OP.subtract)
                zt_sb.append(ztc)
            zm = epool.tile([P, T], F32, name="zm")
            nc.vector.tensor_tensor(zm[:], zt_sb[0][:], zt_sb[1][:], OP.max)
            nc.vector.tensor_tensor(zm[:], zm[:], zt_sb[2][:], OP.max)
            y_sb = []
            for c in range(C):
                yc = epool.tile([P, T], F32, name=f"y{c}")
                nc.vector.tensor_tensor(yc[:], zt_sb[c][:], zm[:], OP.is_equal)
                nc.sync.dma_start(out=y_pl[c], in_=yc[:])
                y_sb.append(yc)
            mm = epool.tile([P, T], F32, name="mmt")
            nc.vector.tensor_tensor(mm[:], y_sb[2][:], y_sb[0][:], OP.subtract)
            nc.sync.dma_start(out=m_pl[:], in_=mm[:])

    nc.finalize()
    return nc


def _build_w2blk(W2):
    # w2blk[c16][(i,l), 3i+c] = W2[c, c16*L + l]
    blk = np.zeros((NCH, 128, 32), np.float32)
    for i in range(R):
        for c in range(C):
            for c16 in range(NCH):
                blk[c16, i * L:(i + 1) * L, C * i + c] = W2[c, c16 * L:(c16 + 1) * L]
    return blk


def _prep_inputs(gene_emb, u, W1, b1, W2):
    geT = np.ascontiguousarray(gene_emb.transpose(0, 2, 1))
    w1sT = np.ascontiguousarray(W1[:, :D].T)
    w1tT = np.ascontiguousarray(W1[:, D:].T)
    w2tp = np.zeros((D, 32), np.float32)
    w2tp[:, :C] = W2.T
    w2blk = _build_w2blk(W2)
    b1col = np.ascontiguousarray(b1.reshape(D, 1))
    mask = np.zeros((P, 1), np.float32)
    for grp in range(NG):
        if GROUP_ENG[grp] != "a":
            mask[grp * R:(grp + 1) * R] = 1.0
    uP = np.ascontiguousarray(u.transpose(3, 0, 1, 2))
    in_maps = []
    for k in range(N_CORES):
        sl = slice(k * S, (k + 1) * S)
        in_maps.append({
            "geT": geT,
            "geS": np.ascontiguousarray(geT[:, :, sl]),
            "w1sT": w1sT,
            "w1tT": w1tT,
            "w2tp": w2tp,
            "w2blk": w2blk,
            "b1c": b1col,
            "hswmask": mask,
            "u_pl": np.ascontiguousarray(uP[:, :, sl, :]).reshape(C, P, T),
        })
    return in_maps


def _gather(results):
    M = np.empty((B, G, G), np.float32)
    y = np.empty((B, G, G, C), np.float32)
    for k in range(N_CORES):
        ypl = results[k]["y_pl"]
        mpl = results[k]["m_pl"]
        for b in range(B):
            M[b, k * S:(k + 1) * S] = mpl[b * S:(b + 1) * S]
            y[b, k * S:(k + 1) * S] = ypl[:, b * S:(b + 1) * S, :].transpose(1, 2, 0)
    return M, y


_prog_cache = {}


def kernel(gene_emb, u, W1, b1, W2, b2, _trace=False, _trace_kwargs=None):
    key = tuple(np.asarray(b2, np.float32).tolist())
    if key not in _prog_cache:
        _prog_cache[key] = build_program(np.asarray(b2, np.float32))
    nc = _prog_cache[key]
    in_maps = _prep_inputs(np.asarray(gene_emb, np.float32),
                           np.asarray(u, np.float32),
                           np.asarray(W1, np.float32),
                           np.asarray(b1, np.float32),
                           np.asarray(W2, np.float32))
    kw = {}
    if _trace:
        kw = dict(trace=True, **(_trace_kwargs or {}))
    res = run_bass_kernel_spmd(nc, in_maps, list(range(N_CORES)), **kw)
    M, y = _gather(res.results)
    if _trace:
        return (M, y), res
    return M, y
